# revision 52
# baseline (speedup 1.0000x reference)
"""Bag-of-words histogram kernel for Trainium2 (Bass/Tile), 8-core data-parallel.

Problem: docs [256, 2048] int32 token ids in [0, 32000) ->
         hist [256, 32000] fp32, hist[b, v] = count(docs[b, :] == v) / 2048.

Algorithm (per core, 32 rows): factor t = 256*hi + lo. For each 128-token
tile build oh_hi[t, hi] and oh_lo[t, lo], then PSUM-accumulate
hist[hi, lo] = oh_hi^T @ oh_lo on the PE over 16 tiles per row.

The build work is spread over THREE engines (the kernel is build-bound;
every compare-style op handles at most 128 tokens, so instruction count is
everything):
  - GPSIMD local_scatter (dst[:]=0; dst[p, idx[p,j]] = payload, per-partition
    indices, executed in Q7-local RAM, ~0.9ns/elem + small fixed cost) is the
    cheapest one-hot builder. 'dg' rows pack ALL 16 hi one-hots of a row in
    ONE call using 127-wide tiles (hi < 125; 16*127 = 2032 < 2046 limit);
    'gs' rows pack lo one-hots 6/6/4 per call (256-wide, idx = lo + 256*(k
    mod 6)). Index tensors are one fused int32-add-with-int16-output per
    group (arithmetic ops cast on output; bit ops do not).
  - DVE tensor_scalar is_equal builds the 'dg' rows' lo one-hots.
  - ACT builds hi for 'gs' rows as a theta code theta[t,j]=sign(hi+.5-j) in
    ONE Sign op; hist[h] = (G[h]-G[h+1]) is decoded with a partition-shifted
    SBUF->SBUF DMA copy + one DVE subtract (engines cannot read across
    partitions; the DMA can).

Scale folding: one side of each product carries 1/SEQ exactly (2^-11, or
2^-12 for gs rows where the +-theta difference restores 2^-11), so PSUM
holds count/2048 exactly and the copyback needs no scaling pass.

Row flavors (tuned so DVE ~ GPSIMD ~ ACT finish together; both probes with
more gs rows regressed via ACT queue head-of-line blocking):
  - dg (20): lo on DVE, hi on GPSIMD (one packed call), ACT copyback.
  - gs (12): lo on GPSIMD (3 calls), hi on ACT Sign, DMA-shift decode.

Sharding: batch axis split 8 ways (32 rows per core), no communication.
"""

import sys

import numpy as np

for _p in ("/opt/trn_rl_repo",):
    if _p not in sys.path:
        sys.path.append(_p)

BATCH = 256
SEQ = 2048
VOCAB = 32000
N_CORES = 8
ROWS = BATCH // N_CORES  # 32 rows per core
P = 128
KT = SEQ // P            # 16 K-tiles per row
NLO = 256                # low-digit bins (t & 255)
NHI = 128                # high-digit compare width (t >> 8 < 125)
NHW = 127                # packed hi-scatter tile width (16*127 < 2046)
SCATTER_TILES = (6, 6, 4)   # k-tiles per local_scatter call (num_elems<2048)

GROUPS = [(0, 4), (4, 16), (16, 32)]  # small head group starts builds early


def _flavor(r):
    if r % 8 in (1, 4, 6):
        return "gs"   # lo GPSIMD, hi ACT Sign (12 rows)
    return "dg"       # lo DVE, hi GPSIMD (20 rows)


FLAVOR = [_flavor(r) for r in range(ROWS)]


def _build_nc():
    from contextlib import ExitStack

    from concourse import bacc, bass, mybir
    from concourse.tile import TileContext

    nc = bacc.Bacc()
    docs = nc.dram_tensor("docs", [ROWS, SEQ], mybir.dt.int32, kind="ExternalInput")
    hist = nc.dram_tensor("hist", [ROWS, VOCAB], mybir.dt.float32, kind="ExternalOutput")

    f32 = mybir.dt.float32
    bf16 = mybir.dt.bfloat16
    i32 = mybir.dt.int32
    i16 = mybir.dt.int16
    Alu = mybir.AluOpType
    Act = mybir.ActivationFunctionType

    with TileContext(nc) as tc, ExitStack() as ctx:
        const_tp = ctx.enter_context(tc.tile_pool(name="const", bufs=1))
        tok_tp = ctx.enter_context(tc.tile_pool(name="tok", bufs=6))
        hilo_tp = ctx.enter_context(tc.tile_pool(name="hilo", bufs=14))
        ohlo_tp = ctx.enter_context(tc.tile_pool(name="ohlo", bufs=14))
        lob6_tp = ctx.enter_context(tc.tile_pool(name="lob6", bufs=6))
        lob4_tp = ctx.enter_context(tc.tile_pool(name="lob4", bufs=3))
        hib8_tp = ctx.enter_context(tc.tile_pool(name="hib8", bufs=8))
        th_tp = ctx.enter_context(tc.tile_pool(name="th", bufs=10))
        gs_tp = ctx.enter_context(tc.tile_pool(name="gs", bufs=6))
        res_tp = ctx.enter_context(tc.tile_pool(name="res", bufs=8))
        psum_tp = ctx.enter_context(tc.tile_pool(name="psum", bufs=8, space="PSUM"))

        # iota constants: value = column index, identical on every partition
        iota_hi = const_tp.tile([P, NHI], bf16)
        nc.gpsimd.iota(iota_hi[:], [[1, NHI]], channel_multiplier=0,
                       allow_small_or_imprecise_dtypes=True)
        iota_lo = const_tp.tile([P, NLO], bf16)
        nc.gpsimd.iota(iota_lo[:], [[1, NLO]], channel_multiplier=0,
                       allow_small_or_imprecise_dtypes=True)
        # scatter-index offsets: 256*(position within each call), replicated
        # along the row dim so the per-group add is a plain slice
        offs = const_tp.tile([P, 16, KT], i32)
        pos = 0
        for nt in SCATTER_TILES:
            nc.gpsimd.iota(offs[:, :, pos:pos + nt], [[0, 16], [NLO, nt]],
                           channel_multiplier=0,
                           allow_small_or_imprecise_dtypes=True)
            pos += nt
        # hi-scatter offsets: 127*(k) - hi < 125 so 127-wide tiles suffice
        # and all 16 fit one local_scatter call (16*127 = 2032 < 2046)
        offs_hi = const_tp.tile([P, 16, KT], i32)
        nc.gpsimd.iota(offs_hi[:], [[0, 16], [NHW, KT]],
                       channel_multiplier=0,
                       allow_small_or_imprecise_dtypes=True)
        data1 = const_tp.tile([P, KT], bf16)
        nc.vector.memset(data1[:], 1.0)
        # per-call scatter payloads (the folded 1/SEQ scale)
        data11 = const_tp.tile([P, 6], bf16)
        nc.vector.memset(data11[:], float(2.0 ** -11))
        data12 = const_tp.tile([P, 6], bf16)
        nc.vector.memset(data12[:], float(2.0 ** -12))

        for (r0, r1) in GROUPS:
            nr = r1 - r0
            # Load rows [r0, r1); partition p holds tokens [16p, 16p+16) of
            # each row (any within-row permutation is histogram-invariant, so
            # a fully contiguous 64B-per-partition-line DMA is used).
            tok_t = tok_tp.tile([P, 16, KT], i32)
            tok = tok_t[:, 0:nr, :]
            src = bass.AP(docs, r0 * SEQ, [[16, P], [SEQ, nr], [1, KT]])
            nc.sync.dma_start(out=tok, in_=src)

            # hi = t >> 8, lo = t & 255; bit-vector ops cannot cast on HW,
            # so shift/and stay int32 and a mult-by-1.0 does the fp32 cast.
            hi_t = hilo_tp.tile([P, 16, KT], i32, tag="hii")
            lo_t = hilo_tp.tile([P, 16, KT], i32, tag="loi")
            hi_i = hi_t[:, 0:nr, :]
            lo_i = lo_t[:, 0:nr, :]
            nc.vector.tensor_scalar(out=hi_i, in0=tok, scalar1=8,
                                    scalar2=None, op0=Alu.logical_shift_right)
            nc.vector.tensor_scalar(out=lo_i, in0=tok, scalar1=255,
                                    scalar2=None, op0=Alu.bitwise_and)
            lo_plt = hilo_tp.tile([P, 16, KT], f32, tag="lo")
            lo_pl = lo_plt[:, 0:nr, :]
            nc.vector.tensor_scalar(out=lo_pl, in0=lo_i, scalar1=1.0,
                                    scalar2=None, op0=Alu.mult)
            # theta bias for hisign rows: hi + 0.5
            hib_t = hilo_tp.tile([P, 16, KT], f32, tag="hib")
            hib = hib_t[:, 0:nr, :]
            nc.vector.tensor_scalar(out=hib, in0=hi_i, scalar1=1.0,
                                    scalar2=0.5, op0=Alu.mult, op1=Alu.add)
            # scatter indices: lo + 256*(k mod 6) as int16 (add casts on out)
            sidx_t = hilo_tp.tile([P, 16, KT], i16, tag="sx16")
            sidx = sidx_t[:, 0:nr, :]
            nc.vector.tensor_tensor(out=sidx, in0=lo_i,
                                    in1=offs[:, 0:nr, :], op=Alu.add)
            # hi-scatter indices: hi + 127*k as int16 (add casts on output)
            hidx_t = hilo_tp.tile([P, 16, KT], i16, tag="hx16")
            hidx = hidx_t[:, 0:nr, :]
            nc.vector.tensor_tensor(out=hidx, in0=hi_i,
                                    in1=offs_hi[:, 0:nr, :], op=Alu.add)

            for rl in range(nr):
                r = r0 + rl
                flavor = FLAVOR[r]
                ps = psum_tp.tile([P, NLO], f32)

                if flavor == "gs":
                    # lo side: packed local_scatter calls (payload 2^-12)
                    lo_slices = [None] * KT
                    pos = 0
                    for nt in SCATTER_TILES:
                        ne = nt * NLO
                        if nt == 6:
                            buf = lob6_tp.tile([P, 6 * NLO], bf16, name="lob6t")
                        else:
                            buf = lob4_tp.tile([P, 4 * NLO], bf16, name="lob4t")
                        nc.gpsimd.local_scatter(
                            buf[:, 0:ne], data12[:, 0:nt],
                            sidx_t[:, rl, pos:pos + nt],
                            channels=P, num_elems=ne, num_idxs=nt)
                        for j in range(nt):
                            lo_slices[pos + j] = buf[:, j * NLO:(j + 1) * NLO]
                        pos += nt
                else:
                    # hi side: ONE packed local_scatter call (payload 1.0)
                    hbuf = hib8_tp.tile([P, KT * NHW], bf16, name="hib8t")
                    nc.gpsimd.local_scatter(
                        hbuf[:], data1[:], hidx_t[:, rl, :],
                        channels=P, num_elems=KT * NHW, num_idxs=KT)
                    hi_slices = [hbuf[:, j * NHW:(j + 1) * NHW]
                                 for j in range(KT)]

                for k in range(KT):
                    if flavor == "gs":
                        lhs = th_tp.tile([P, NHI], bf16, tag="thh")
                        nc.scalar.activation(
                            out=lhs[:], in_=iota_hi[:], func=Act.Sign,
                            bias=hib_t[:, rl, k:k + 1], scale=-1.0)
                        rhs = lo_slices[k]
                    else:
                        lhs = hi_slices[k]
                        oh_lo = ohlo_tp.tile([P, NLO], bf16)
                        nc.vector.tensor_scalar(
                            out=oh_lo[:], in0=iota_lo[:],
                            scalar1=lo_plt[:, rl, k:k + 1],
                            scalar2=float(2.0 ** -11),
                            op0=Alu.is_equal, op1=Alu.mult)
                        rhs = oh_lo[:]
                    if flavor == "gs":
                        nc.tensor.matmul(out=ps[:], lhsT=lhs[:], rhs=rhs,
                                         start=(k == 0), stop=(k == KT - 1))
                    else:
                        nc.tensor.matmul(out=ps[0:NHW, :], lhsT=lhs, rhs=rhs,
                                         start=(k == 0), stop=(k == KT - 1))

                if flavor == "gs":
                    # hist[h, l] = G[h, l] - G[h+1, l]: the partition shift
                    # is done by an SBUF->SBUF DMA (engines cannot cross
                    # partitions), then one DVE subtract.
                    gs = gs_tp.tile([P, NLO], f32, tag="gs2")
                    nc.scalar.copy(out=gs[:, :], in_=ps[:, :])
                    gsh = gs_tp.tile([P, NLO], f32, tag="gsh")
                    nc.sync.dma_start(out=gsh[0:126, :], in_=gs[1:127, :])
                    res = res_tp.tile([P, NLO], f32, tag="ress")
                    nc.vector.tensor_tensor(out=res[0:126, :], in0=gs[0:126, :],
                                            in1=gsh[0:126, :],
                                            op=Alu.subtract)
                else:
                    res = res_tp.tile([P, NLO], f32, tag="resp")
                    nc.scalar.copy(out=res[:], in_=ps[:])
                nc.sync.dma_start(
                    out=hist[r].rearrange("(h l) -> h l", l=NLO),
                    in_=res[:VOCAB // NLO, :])
    nc.compile()
    return nc


_NC_CACHE = None


def _get_nc():
    global _NC_CACHE
    if _NC_CACHE is None:
        _NC_CACHE = _build_nc()
    return _NC_CACHE


def run_sharded(docs: np.ndarray, trace: bool = False):
    """Run the 8-core SPMD kernel. Returns (full_output, BassKernelResults)."""
    from concourse.bass_utils import run_bass_kernel_spmd

    docs = np.ascontiguousarray(np.asarray(docs, dtype=np.int32))
    assert docs.shape == (BATCH, SEQ), docs.shape
    shards = docs.reshape(N_CORES, ROWS, SEQ)
    in_maps = [{"docs": shards[i]} for i in range(N_CORES)]
    res = run_bass_kernel_spmd(_get_nc(), in_maps, core_ids=list(range(N_CORES)),
                               trace=trace)
    out = np.concatenate([res.results[i]["hist"] for i in range(N_CORES)], axis=0)
    return out, res


def kernel(docs: np.ndarray) -> np.ndarray:
    out, _ = run_sharded(docs, trace=False)
    return out


# revision 57
# speedup vs baseline: 1.1868x; 1.1868x over previous
"""Bag-of-words histogram kernel for Trainium2 (Bass/Tile), 8-core data-parallel.

Problem: docs [256, 2048] int32 token ids in [0, 32000) ->
         hist [256, 32000] fp32, hist[b, v] = count(docs[b, :] == v) / 2048.

Algorithm (per core, 32 rows): factor t = 256*hi + lo. For each 128-token
tile build oh_hi[t, hi] and oh_lo[t, lo], then PSUM-accumulate
hist[hi, lo] = oh_hi^T @ oh_lo on the PE over 16 tiles per row.

The build work is spread over THREE engines (the kernel is build-bound;
every compare-style op handles at most 128 tokens, so instruction count is
everything):
  - GPSIMD local_scatter (dst[:]=0; dst[p, idx[p,j]] = payload, per-partition
    indices, executed in Q7-local RAM, ~0.9ns/elem + small fixed cost) is the
    cheapest one-hot builder. 'dg' rows pack ALL 16 hi one-hots of a row in
    ONE call using 127-wide tiles (hi < 125; 16*127 = 2032 < 2046 limit);
    'gs' rows pack lo one-hots 6/6/4 per call (256-wide, idx = lo + 256*(k
    mod 6)). Index tensors are one fused int32-add-with-int16-output per
    group (arithmetic ops cast on output; bit ops do not).
  - DVE tensor_scalar is_equal builds the 'dg' rows' lo one-hots.
  - ACT builds hi for 'gs' rows as a theta code theta[t,j]=sign(hi+.5-j) in
    ONE Sign op; hist[h] = (G[h]-G[h+1]) is decoded with a partition-shifted
    SBUF->SBUF DMA copy + one DVE subtract (engines cannot read across
    partitions; the DMA can).

Scale folding: one side of each product carries 1/SEQ exactly (2^-11, or
2^-12 for gs rows where the +-theta difference restores 2^-11), so PSUM
holds count/2048 exactly and the copyback needs no scaling pass.

Row flavors (tuned so DVE ~ GPSIMD ~ ACT finish together; both probes with
more gs rows regressed via ACT queue head-of-line blocking):
  - dg (20): lo on DVE, hi on GPSIMD (one packed call), ACT copyback.
  - gs (12): lo on GPSIMD (3 calls), hi on ACT Sign, DMA-shift decode.

Sharding: batch axis split 8 ways (32 rows per core), no communication.
"""

import sys

import numpy as np

for _p in ("/opt/trn_rl_repo",):
    if _p not in sys.path:
        sys.path.append(_p)

BATCH = 256
SEQ = 2048
VOCAB = 32000
N_CORES = 8
ROWS = BATCH // N_CORES  # 32 rows per core
P = 128
KT = SEQ // P            # 16 K-tiles per row
NLO = 256                # low-digit bins (t & 255)
NHI = 128                # high-digit compare width (t >> 8 < 125)
NHW = 127                # packed hi-scatter tile width (16*127 < 2046)
SCATTER_TILES = (6, 6, 4)   # k-tiles per local_scatter call (num_elems<2048)

GROUPS = [(0, 4), (4, 16), (16, 32)]  # small head group starts builds early


def _flavor(r):
    if r % 8 in (1, 4, 6):
        return "gs"   # lo GPSIMD, hi ACT Sign (12 rows)
    return "dg"       # lo DVE, hi GPSIMD (20 rows)


FLAVOR = [_flavor(r) for r in range(ROWS)]


def _build_nc():
    from contextlib import ExitStack

    from concourse import bacc, bass, mybir
    from concourse.tile import TileContext

    nc = bacc.Bacc()
    docs = nc.dram_tensor("docs", [ROWS, SEQ], mybir.dt.int32, kind="ExternalInput")
    hist = nc.dram_tensor("hist", [ROWS, VOCAB], mybir.dt.float32, kind="ExternalOutput")

    f32 = mybir.dt.float32
    bf16 = mybir.dt.bfloat16
    i32 = mybir.dt.int32
    i16 = mybir.dt.int16
    Alu = mybir.AluOpType
    Act = mybir.ActivationFunctionType

    with TileContext(nc) as tc, ExitStack() as ctx:
        const_tp = ctx.enter_context(tc.tile_pool(name="const", bufs=1))
        tok_tp = ctx.enter_context(tc.tile_pool(name="tok", bufs=6))
        hilo_tp = ctx.enter_context(tc.tile_pool(name="hilo", bufs=14))
        ohlo_tp = ctx.enter_context(tc.tile_pool(name="ohlo", bufs=14))
        lob6_tp = ctx.enter_context(tc.tile_pool(name="lob6", bufs=6))
        lob4_tp = ctx.enter_context(tc.tile_pool(name="lob4", bufs=3))
        hib8_tp = ctx.enter_context(tc.tile_pool(name="hib8", bufs=8))
        th_tp = ctx.enter_context(tc.tile_pool(name="th", bufs=10))
        gs_tp = ctx.enter_context(tc.tile_pool(name="gs", bufs=6))
        res_tp = ctx.enter_context(tc.tile_pool(name="res", bufs=8))
        psum_tp = ctx.enter_context(tc.tile_pool(name="psum", bufs=8, space="PSUM"))

        # iota constants: value = column index, identical on every partition
        iota_hi = const_tp.tile([P, NHI], bf16)
        nc.gpsimd.iota(iota_hi[:], [[1, NHI]], channel_multiplier=0,
                       allow_small_or_imprecise_dtypes=True)
        iota_lo = const_tp.tile([P, NLO], bf16)
        nc.gpsimd.iota(iota_lo[:], [[1, NLO]], channel_multiplier=0,
                       allow_small_or_imprecise_dtypes=True)
        # scatter-index offsets: 256*(position within each call), replicated
        # along the row dim so the per-group add is a plain slice
        offs = const_tp.tile([P, 16, KT], i32)
        pos = 0
        for nt in SCATTER_TILES:
            nc.gpsimd.iota(offs[:, :, pos:pos + nt], [[0, 16], [NLO, nt]],
                           channel_multiplier=0,
                           allow_small_or_imprecise_dtypes=True)
            pos += nt
        # hi-scatter offsets: 127*(k) - hi < 125 so 127-wide tiles suffice
        # and all 16 fit one local_scatter call (16*127 = 2032 < 2046)
        offs_hi = const_tp.tile([P, 16, KT], i32)
        nc.gpsimd.iota(offs_hi[:], [[0, 16], [NHW, KT]],
                       channel_multiplier=0,
                       allow_small_or_imprecise_dtypes=True)
        data1 = const_tp.tile([P, KT], bf16)
        nc.vector.memset(data1[:], 1.0)
        # per-call scatter payloads (the folded 1/SEQ scale)
        data11 = const_tp.tile([P, 6], bf16)
        nc.vector.memset(data11[:], float(2.0 ** -11))
        data12 = const_tp.tile([P, 6], bf16)
        nc.vector.memset(data12[:], float(2.0 ** -12))

        for (r0, r1) in GROUPS:
            nr = r1 - r0
            # Load rows [r0, r1); partition p holds tokens [16p, 16p+16) of
            # each row (any within-row permutation is histogram-invariant, so
            # a fully contiguous 64B-per-partition-line DMA is used).
            tok_t = tok_tp.tile([P, 16, KT], i32)
            tok = tok_t[:, 0:nr, :]
            src = bass.AP(docs, r0 * SEQ, [[16, P], [SEQ, nr], [1, KT]])
            nc.sync.dma_start(out=tok, in_=src)

            # hi = t >> 8, lo = t & 255; bit-vector ops cannot cast on HW,
            # so shift/and stay int32 and a mult-by-1.0 does the fp32 cast.
            hi_t = hilo_tp.tile([P, 16, KT], i32, tag="hii")
            lo_t = hilo_tp.tile([P, 16, KT], i32, tag="loi")
            hi_i = hi_t[:, 0:nr, :]
            lo_i = lo_t[:, 0:nr, :]
            nc.vector.tensor_scalar(out=hi_i, in0=tok, scalar1=8,
                                    scalar2=None, op0=Alu.logical_shift_right)
            nc.vector.tensor_scalar(out=lo_i, in0=tok, scalar1=255,
                                    scalar2=None, op0=Alu.bitwise_and)
            lo_plt = hilo_tp.tile([P, 16, KT], f32, tag="lo")
            lo_pl = lo_plt[:, 0:nr, :]
            nc.vector.tensor_scalar(out=lo_pl, in0=lo_i, scalar1=1.0,
                                    scalar2=None, op0=Alu.mult)
            # theta bias for hisign rows: hi + 0.5
            hib_t = hilo_tp.tile([P, 16, KT], f32, tag="hib")
            hib = hib_t[:, 0:nr, :]
            nc.vector.tensor_scalar(out=hib, in0=hi_i, scalar1=1.0,
                                    scalar2=0.5, op0=Alu.mult, op1=Alu.add)
            # scatter indices: lo + 256*(k mod 6) as int16 (add casts on out)
            sidx_t = hilo_tp.tile([P, 16, KT], i16, tag="sx16")
            sidx = sidx_t[:, 0:nr, :]
            nc.vector.tensor_tensor(out=sidx, in0=lo_i,
                                    in1=offs[:, 0:nr, :], op=Alu.add)
            # hi-scatter indices: hi + 127*k as int16 (add casts on output)
            hidx_t = hilo_tp.tile([P, 16, KT], i16, tag="hx16")
            hidx = hidx_t[:, 0:nr, :]
            nc.vector.tensor_tensor(out=hidx, in0=hi_i,
                                    in1=offs_hi[:, 0:nr, :], op=Alu.add)

            for rl in range(nr):
                r = r0 + rl
                flavor = FLAVOR[r]
                ps = psum_tp.tile([P, NLO], f32)

                if flavor == "gs":
                    # lo side: packed local_scatter calls (payload 2^-12)
                    lo_slices = [None] * KT
                    pos = 0
                    for nt in SCATTER_TILES:
                        ne = nt * NLO
                        if nt == 6:
                            buf = lob6_tp.tile([P, 6 * NLO], bf16, name="lob6t")
                        else:
                            buf = lob4_tp.tile([P, 4 * NLO], bf16, name="lob4t")
                        nc.gpsimd.local_scatter(
                            buf[:, 0:ne], data12[:, 0:nt],
                            sidx_t[:, rl, pos:pos + nt],
                            channels=P, num_elems=ne, num_idxs=nt)
                        for j in range(nt):
                            lo_slices[pos + j] = buf[:, j * NLO:(j + 1) * NLO]
                        pos += nt
                else:
                    # hi side: ONE packed local_scatter call (payload 1.0)
                    hbuf = hib8_tp.tile([P, KT * NHW], bf16, name="hib8t")
                    nc.gpsimd.local_scatter(
                        hbuf[:], data1[:], hidx_t[:, rl, :],
                        channels=P, num_elems=KT * NHW, num_idxs=KT)
                    hi_slices = [hbuf[:, j * NHW:(j + 1) * NHW]
                                 for j in range(KT)]

                for k in range(KT):
                    if flavor == "gs":
                        lhs = th_tp.tile([P, NHI], bf16, tag="thh")
                        nc.scalar.activation(
                            out=lhs[:], in_=iota_hi[:], func=Act.Sign,
                            bias=hib_t[:, rl, k:k + 1], scale=-1.0)
                        rhs = lo_slices[k]
                    else:
                        lhs = hi_slices[k]
                        oh_lo = ohlo_tp.tile([P, NLO], bf16)
                        nc.vector.tensor_scalar(
                            out=oh_lo[:], in0=iota_lo[:],
                            scalar1=lo_plt[:, rl, k:k + 1],
                            scalar2=float(2.0 ** -11),
                            op0=Alu.is_equal, op1=Alu.mult)
                        rhs = oh_lo[:]
                    if flavor == "gs":
                        nc.tensor.matmul(out=ps[:], lhsT=lhs[:], rhs=rhs,
                                         start=(k == 0), stop=(k == KT - 1))
                    else:
                        nc.tensor.matmul(out=ps[0:NHW, :], lhsT=lhs, rhs=rhs,
                                         start=(k == 0), stop=(k == KT - 1))

                if flavor == "gs":
                    # hist[h, l] = G[h, l] - G[h+1, l]: the partition shift
                    # is done by an SBUF->SBUF DMA (engines cannot cross
                    # partitions), then one DVE subtract.
                    gs = gs_tp.tile([P, NLO], f32, tag="gs2")
                    nc.scalar.copy(out=gs[:, :], in_=ps[:, :])
                    gsh = gs_tp.tile([P, NLO], f32, tag="gsh")
                    nc.sync.dma_start(out=gsh[0:126, :], in_=gs[1:127, :])
                    res = res_tp.tile([P, NLO], f32, tag="ress")
                    nc.vector.tensor_tensor(out=res[0:126, :], in0=gs[0:126, :],
                                            in1=gsh[0:126, :],
                                            op=Alu.subtract)
                else:
                    res = res_tp.tile([P, NLO], f32, tag="resp")
                    nc.scalar.copy(out=res[:], in_=ps[:])
                nc.sync.dma_start(
                    out=hist[r].rearrange("(h l) -> h l", l=NLO),
                    in_=res[:VOCAB // NLO, :])
    nc.compile()
    return nc


_NC_CACHE = None


def _get_nc():
    global _NC_CACHE
    if _NC_CACHE is None:
        _NC_CACHE = _build_nc()
    return _NC_CACHE


def run_sharded(docs: np.ndarray, trace: bool = False):
    """Run the 8-core SPMD kernel. Returns (full_output, BassKernelResults)."""
    from concourse.bass_utils import run_bass_kernel_spmd

    docs = np.ascontiguousarray(np.asarray(docs, dtype=np.int32))
    assert docs.shape == (BATCH, SEQ), docs.shape
    shards = docs.reshape(N_CORES, ROWS, SEQ)
    in_maps = [{"docs": shards[i]} for i in range(N_CORES)]
    res = run_bass_kernel_spmd(_get_nc(), in_maps, core_ids=list(range(N_CORES)),
                               trace=trace)
    out = np.concatenate([res.results[i]["hist"] for i in range(N_CORES)], axis=0)
    return out, res


def kernel(docs: np.ndarray) -> np.ndarray:
    out, _ = run_sharded(docs, trace=False)
    return out
